# revision 9
# baseline (speedup 1.0000x reference)
"""Trainium2 Bass kernel for nn_EnhancedAttention (16-head attention with a
full [H,S,S] additive position bias), sharded 2-heads-per-core over 8 cores.

v3 (vs v2 baseline, 333us):
  - HAM p-state fix: the attention phase starved the PE every kt-tile
    (ACT exp 1147ns > PE 852ns), keeping the PE clock-gated at 1.2GHz.
    Now half the kt-tiles "inject" the raw position bias into PSUM via an
    identity matmul (PE: +426ns/tile) so exp(s+rel) needs no DVE multiply,
    while the other half keep the exp(rel) DVE-multiply path. Average PE
    work/kt (~1170ns) slightly exceeds ACT (1147ns), keeping the PE
    saturated and warm (2.4GHz).
  - softmax denominator reciprocal moved from ACT (Ln+Exp, ~23us) to a
    single DVE custom op (reciprocal_approx_fast), broadcast in fp32.
  - PSUM: psS bufs=3 x [128,1024] (6 banks) + psC bufs=2 (2 banks) = 8.
"""

import numpy as np

import concourse.bacc as bacc
import concourse.tile as tile
import concourse.mybir as mybir
from concourse.bass_utils import run_bass_kernel_spmd
from concourse.masks import make_identity

FP16 = mybir.dt.float16
FP32 = mybir.dt.float32
Exp = mybir.ActivationFunctionType.Exp
Ln = mybir.ActivationFunctionType.Ln
Copy = mybir.ActivationFunctionType.Copy
MULT = mybir.AluOpType.mult

P = 128
B, S, D = 2, 2048, 1024
H, HD = 16, 64
NCORES = 8
HPC = H // NCORES          # heads per core = 2
DT = D // P                # 8 d-tiles
ST = S // P                # 16 s-tiles (k tiles / out row tiles)
QC = 4                     # q chunks
QCW = S // QC              # 512
VSTRIDE = 2 * (HD + 1)     # 130: [h0 v (64) | ones | h1 v (64) | ones]


_NC_CACHE = {}


class _Bacc(bacc.Bacc):
    """Pin activations to natural_log_exp_and_others (Copy/Exp only now)."""

    def insert_act_table_loads(self):
        import bass_rust as _bass_rust
        import concourse.mybir as _mybir
        from concourse.hw_specs import get_activation_tables
        has_activation = any(
            isinstance(i, _mybir.InstActivation)
            for b in self.main_func.blocks
            for i in b.instructions
        )
        if not has_activation:
            return
        tables = []
        for name, fns in get_activation_tables(self.m.arch).items():
            tables.append((name, fns if name == "natural_log_exp_and_others" else type(fns)()))
        _bass_rust.insert_act_table_loads(self, tables)


def _build_nc():
    nc = _Bacc("TRN2", target_bir_lowering=False)

    hT = nc.dram_tensor("hT", [B, P, DT, S], FP16, kind="ExternalInput")
    w3 = nc.dram_tensor("w3", [P, 3 * DT * P], FP16, kind="ExternalInput")
    eb = nc.dram_tensor("eb", [HPC, S, S], FP16, kind="ExternalInput")
    woT = nc.dram_tensor("woT", [P, D], FP16, kind="ExternalInput")
    outp = nc.dram_tensor("outp", [B, S, D], FP16, kind="ExternalOutput")

    with tile.TileContext(nc) as tc:
        # ---- persistent tiles ----
        persist = tc.alloc_tile_pool(name="persist", bufs=1)
        qT_sb = [persist.tile([P, S], FP16, tag=f"qT{b}", name=f"qT{b}") for b in range(B)]
        kT_sb = [persist.tile([P, S], FP16, tag=f"kT{b}", name=f"kT{b}") for b in range(B)]
        ctxn = [persist.tile([P, S], FP16, tag=f"ctxn{b}", name=f"ctxn{b}") for b in range(B)]
        v_all = persist.tile([P, B * ST * VSTRIDE], FP16, tag="v_all", name="v_all")
        w_sb = persist.tile([P, 3 * DT * P], FP16, tag="w_sb", name="w_sb")
        woT_sb = persist.tile([P, D], FP16, tag="woT_sb", name="woT_sb")
        ident = persist.tile([P, P], FP16, tag="ident", name="ident")

        make_identity(nc, ident[:])
        nc.any.memset(v_all[:], 1.0)  # ones columns survive the v copies
        nc.sync.dma_start(woT_sb[:], woT[:])
        nc.sync.dma_start(w_sb[:], w3[:])

        # ---- phase P: projections + v transpose ----
        with (
            tc.tile_pool(name="hp", bufs=2) as hp,
            tc.tile_pool(name="vt", bufs=8) as vtp,
            tc.tile_pool(name="psP", bufs=6, space="PSUM") as psP,
            tc.tile_pool(name="psT", bufs=2, space="PSUM") as psT,
        ):
            for b in range(B):
                h_all = hp.tile([P, DT * S], FP16, tag="h", name=f"h_{b}")
                for dt in range(DT):
                    nc.sync.dma_start(h_all[:, dt * S:(dt + 1) * S], hT[b, :, dt])
                h_sb = [h_all[:, dt * S:(dt + 1) * S] for dt in range(DT)]
                vT_tiles = []
                for p in range(3):
                    ps_qc = [
                        psP.tile([P, QCW], FP32, tag="pj", name=f"pj_{b}_{p}_{qc}")
                        for qc in range(QC)
                    ]
                    for dt in range(DT):
                        for qc in range(QC):
                            nc.tensor.matmul(
                                ps_qc[qc][:],
                                w_sb[:, (p * DT + dt) * P:(p * DT + dt + 1) * P],
                                h_sb[dt][:, qc * QCW:(qc + 1) * QCW],
                                start=(dt == 0), stop=(dt == DT - 1),
                            )
                    for qc in range(QC):
                        if p == 0:
                            nc.scalar.activation(
                                qT_sb[b][:, qc * QCW:(qc + 1) * QCW], ps_qc[qc][:],
                                Copy, scale=1.0 / np.sqrt(HD),
                            )
                        elif p == 1:
                            nc.scalar.activation(
                                kT_sb[b][:, qc * QCW:(qc + 1) * QCW], ps_qc[qc][:], Copy)
                        else:
                            vt = vtp.tile([P, QCW], FP16, tag="v", name=f"vt_{b}_{qc}")
                            nc.vector.tensor_copy(out=vt[:], in_=ps_qc[qc][:])
                            vT_tiles.append(vt)
                # transpose vT [ch, s] -> v [s, ch] in 128x128 blocks
                for st in range(ST):
                    tp = psT.tile([P, P], FP16, tag="tr", name=f"tr_{b}_{st}")
                    src = vT_tiles[st // 4]
                    nc.tensor.transpose(tp[:], src[:, (st % 4) * P:(st % 4 + 1) * P], ident[:])
                    base = (b * ST + st) * VSTRIDE
                    nc.vector.tensor_copy(out=v_all[:, base:base + HD], in_=tp[:, 0:HD])
                    nc.vector.tensor_copy(
                        out=v_all[:, base + HD + 1:base + 2 * HD + 1], in_=tp[:, HD:2 * HD])

        # ---- phase A: attention ----
        with (
            tc.tile_pool(name="bias", bufs=ST + 8) as bp,
            tc.tile_pool(name="pr", bufs=8) as prp,
            tc.tile_pool(name="sm", bufs=4) as smp,
            tc.tile_pool(name="psS", bufs=3, space="PSUM") as psS,
            tc.tile_pool(name="psC", bufs=2, space="PSUM") as psC,
        ):
            for h in range(HPC):
                eb_sb = {}
                for kt in range(ST):
                    t = bp.tile([P, S], FP16, tag="eb", name=f"eb_{h}_{kt}")
                    nc.sync.dma_start(t[:], eb[h, kt * P:(kt + 1) * P, :])
                    eb_sb[kt] = t
                hs = slice(h * HD, (h + 1) * HD)
                for qc in range(QC):
                    ctx_ps = [
                        psC.tile([P, QCW], FP32, tag="c", name=f"ctx_{h}_{qc}_{b}")
                        for b in range(B)
                    ]
                    for kt in range(ST):
                        s_ps = psS.tile([P, 2 * QCW], FP32, tag="s", name=f"s_{h}_{qc}_{kt}")
                        ebs = eb_sb[kt][:, qc * QCW:(qc + 1) * QCW]
                        for b in range(B):
                            nc.tensor.matmul(
                                s_ps[:, b * QCW:(b + 1) * QCW],
                                ident[:], ebs,
                                start=True, stop=False,
                            )
                            nc.tensor.matmul(
                                s_ps[:, b * QCW:(b + 1) * QCW],
                                kT_sb[b][hs, kt * P:(kt + 1) * P],
                                qT_sb[b][hs, qc * QCW:(qc + 1) * QCW],
                                start=False, stop=True,
                            )
                        pr = prp.tile([P, 2 * QCW], FP16, tag="p", name=f"p_{h}_{qc}_{kt}")
                        nc.scalar.activation(pr[:], s_ps[:], Exp)
                        for b in range(B):
                            vbase = (b * ST + kt) * VSTRIDE + h * (HD + 1)
                            nc.tensor.matmul(
                                ctx_ps[b][0:HD + 1, :],
                                v_all[:, vbase:vbase + HD + 1],
                                pr[:, b * QCW:(b + 1) * QCW],
                                start=(kt == 0), stop=(kt == ST - 1),
                            )
                    for b in range(B):
                        # evacuate PSUM first so psC recycles fast, then
                        # normalize entirely on DVE + GpSimd (ACT stays free).
                        # (recip needs its input at partition base 0.)
                        dn = smp.tile([1, QCW], FP32, tag="dn", name=f"dn_{h}_{qc}_{b}")
                        nc.vector.tensor_copy(out=dn[:], in_=ctx_ps[b][HD:HD + 1, :])
                        cs = smp.tile([HD, QCW], FP32, tag="cs", name=f"cs_{h}_{qc}_{b}")
                        nc.vector.tensor_copy(out=cs[:], in_=ctx_ps[b][0:HD, :])
                        rcp = smp.tile([1, QCW], FP32, tag="rc", name=f"rc_{h}_{qc}_{b}")
                        nc.vector.reciprocal_approx_fast(out=rcp[:], in_=dn[:])
                        bc = smp.tile([HD, QCW], FP32, tag="bc", name=f"bcs_{h}_{qc}_{b}")
                        nc.gpsimd.partition_broadcast(bc[:], rcp[:])
                        nc.vector.tensor_tensor(
                            ctxn[b][hs, qc * QCW:(qc + 1) * QCW],
                            cs[:], bc[:], MULT)

        # ---- phase O: output projection (both heads, K=128) ----
        with (
            tc.tile_pool(name="op", bufs=4) as op,
            tc.tile_pool(name="psO", bufs=3, space="PSUM") as psO,
        ):
            for b in range(B):
                for st in range(ST):
                    o_ps = psO.tile([P, D], FP32, tag="o", name=f"o_{b}_{st}")
                    for ec in range(2):
                        nc.tensor.matmul(
                            o_ps[:, ec * QCW:(ec + 1) * QCW],
                            ctxn[b][:, st * P:(st + 1) * P],
                            woT_sb[:, ec * QCW:(ec + 1) * QCW],
                            start=True, stop=True,
                        )
                    o_sb = op.tile([P, D], FP16, tag="ot", name=f"ot_{b}_{st}")
                    if st % 2 == 0:
                        nc.scalar.activation(o_sb[:], o_ps[:], Copy)
                    else:
                        nc.vector.tensor_copy(out=o_sb[:], in_=o_ps[:])
                    nc.sync.dma_start(outp[b, st * P:(st + 1) * P, :], o_sb[:])

        persist.release()

    nc.finalize()
    return nc


def _numpy_reference(hidden_states, attention_mask, relative_position,
                     Wq, bq, Wk, bk, Wv, bv, Wo, bo):
    Bn, Sn, Dn = hidden_states.shape
    Hn = relative_position.shape[1]
    hd = Dn // Hn
    x = hidden_states.astype(np.float64)

    def heads(t):
        return t.reshape(Bn, Sn, Hn, hd).transpose(0, 2, 1, 3)

    q = heads(x @ Wq.T.astype(np.float64) + bq)
    k = heads(x @ Wk.T.astype(np.float64) + bk)
    v = heads(x @ Wv.T.astype(np.float64) + bv)
    s = np.einsum("bhqd,bhkd->bhqk", q, k) / np.sqrt(hd)
    s = s + relative_position.astype(np.float64) + attention_mask.astype(np.float64)
    s = s - s.max(axis=-1, keepdims=True)
    p = np.exp(s)
    p /= p.sum(axis=-1, keepdims=True)
    ctx = np.einsum("bhqk,bhkd->bhqd", p, v)
    ctx = ctx.transpose(0, 2, 1, 3).reshape(Bn, Sn, Dn)
    return (ctx @ Wo.T.astype(np.float64) + bo).astype(np.float32)


def kernel(hidden_states, attention_mask, relative_position,
           Wq, bq, Wk, bk, Wv, bv, Wo, bo):
    hidden_states = np.asarray(hidden_states)
    attention_mask = np.asarray(attention_mask)
    relative_position = np.asarray(relative_position)
    Wq, bq = np.asarray(Wq), np.asarray(bq)
    Wk, bk = np.asarray(Wk), np.asarray(bk)
    Wv, bv = np.asarray(Wv), np.asarray(bv)
    Wo, bo = np.asarray(Wo), np.asarray(bo)

    # The device program folds the (always-zero) mask and qkv biases away;
    # fall back to a plain numpy path if they are ever nonzero.
    if (np.any(attention_mask) or np.any(bq) or np.any(bk) or np.any(bv)
            or hidden_states.shape != (B, S, D)):
        return _numpy_reference(hidden_states, attention_mask, relative_position,
                                Wq, bq, Wk, bk, Wv, bv, Wo, bo)

    if "nc" not in _NC_CACHE:
        _NC_CACHE["nc"] = _build_nc()
    nc = _NC_CACHE["nc"]

    hT = np.ascontiguousarray(
        hidden_states.transpose(0, 2, 1).reshape(B, DT, P, S).transpose(0, 2, 1, 3)
    ).astype(np.float16)  # [B, 128, dt, S]
    rel = relative_position[0]  # [H, S, S]

    in_maps = []
    for c in range(NCORES):
        sl = slice(c * HPC * HD, (c + 1) * HPC * HD)
        heads = rel[c * HPC:(c + 1) * HPC]  # [HPC, S, S] (q, k)
        ebmix = np.ascontiguousarray(heads.transpose(0, 2, 1)).astype(np.float16)
        w3 = np.ascontiguousarray(
            np.stack([Wq[sl].T, Wk[sl].T, Wv[sl].T])       # [3, D, 128]
            .reshape(3, DT, P, P).transpose(2, 0, 1, 3)     # [128, 3, dt, 128]
            .reshape(P, 3 * DT * P)).astype(np.float16)
        woT = np.ascontiguousarray(Wo[:, sl].T).astype(np.float16)
        in_maps.append({"hT": hT, "w3": w3, "eb": ebmix, "woT": woT})

    res = run_bass_kernel_spmd(nc, in_maps, core_ids=list(range(NCORES)))
    _NC_CACHE["last_results"] = res

    out = np.zeros((B, S, D), np.float32)
    for c in range(NCORES):
        out += res.results[c]["outp"].astype(np.float32)
    out += bo.astype(np.float32)
    return out


# revision 28
# speedup vs baseline: 1.2262x; 1.2262x over previous
"""Trainium2 Bass kernel for nn_EnhancedAttention (16-head attention with a
full [H,S,S] additive position bias), sharded 2-heads-per-core over 8 cores.

v3 (vs v2 baseline, 333us):
  - HAM p-state fix: the attention phase starved the PE every kt-tile
    (ACT exp 1147ns > PE 852ns), keeping the PE clock-gated at 1.2GHz.
    Now half the kt-tiles "inject" the raw position bias into PSUM via an
    identity matmul (PE: +426ns/tile) so exp(s+rel) needs no DVE multiply,
    while the other half keep the exp(rel) DVE-multiply path. Average PE
    work/kt (~1170ns) slightly exceeds ACT (1147ns), keeping the PE
    saturated and warm (2.4GHz).
  - softmax denominator reciprocal moved from ACT (Ln+Exp, ~23us) to a
    single DVE custom op (reciprocal_approx_fast), broadcast in fp32.
  - PSUM: psS bufs=3 x [128,1024] (6 banks) + psC bufs=2 (2 banks) = 8.
"""

import numpy as np

import concourse.bacc as bacc
import concourse.tile as tile
import concourse.mybir as mybir
from concourse.bass_utils import run_bass_kernel_spmd
from concourse.masks import make_identity

FP16 = mybir.dt.float16
FP32 = mybir.dt.float32
FP8 = mybir.dt.float8e4
DR = mybir.MatmulPerfMode.DoubleRow
Exp = mybir.ActivationFunctionType.Exp
Ln = mybir.ActivationFunctionType.Ln
Copy = mybir.ActivationFunctionType.Copy
MULT = mybir.AluOpType.mult

P = 128
B, S, D = 2, 2048, 1024
H, HD = 16, 64
NCORES = 8
HPC = H // NCORES          # heads per core = 2
DT = D // P                # 8 d-tiles
ST = S // P                # 16 s-tiles (k tiles / out row tiles)
QC = 4                     # q chunks
QCW = S // QC              # 512
VSTRIDE = 2 * (HD + 1)     # 130: [h0 v (64) | ones | h1 v (64) | ones]


_NC_CACHE = {}


class _Bacc(bacc.Bacc):
    """Pin activations to natural_log_exp_and_others (Copy/Exp only now)."""

    def insert_act_table_loads(self):
        import bass_rust as _bass_rust
        import concourse.mybir as _mybir
        from concourse.hw_specs import get_activation_tables
        has_activation = any(
            isinstance(i, _mybir.InstActivation)
            for b in self.main_func.blocks
            for i in b.instructions
        )
        if not has_activation:
            return
        tables = []
        for name, fns in get_activation_tables(self.m.arch).items():
            tables.append((name, fns if name == "natural_log_exp_and_others" else type(fns)()))
        _bass_rust.insert_act_table_loads(self, tables)


def _build_nc():
    nc = _Bacc("TRN2", target_bir_lowering=False)

    hT = nc.dram_tensor("hT", [B, P, DT, S], FP16, kind="ExternalInput")
    w3 = nc.dram_tensor("w3", [P, 3 * DT * P], FP16, kind="ExternalInput")
    # rel bias packed for fp8 DoubleRow hi+lo injects: plane 0 = fp8(rel),
    # plane 1 = fp8(rel - plane0); the DoubleRow identity sums both planes,
    # recovering ~fp16 precision at fp8-DoubleRow speed (0.5 cyc/col).
    eb8 = nc.dram_tensor("eb8", [HPC, ST, P, 2, S], FP8, kind="ExternalInput")
    id8 = nc.dram_tensor("id8", [P, 2, P], FP8, kind="ExternalInput")
    woT = nc.dram_tensor("woT", [P, D], FP16, kind="ExternalInput")
    outp = nc.dram_tensor("outp", [B, S, D], FP16, kind="ExternalOutput")

    with tile.TileContext(nc) as tc:
        # ---- persistent tiles ----
        persist = tc.alloc_tile_pool(name="persist", bufs=1)
        # per-(b, h) zero-padded q: the other head's 64 rows stay 0 so the
        # scores matmul can contract over the full 128 partitions (keeps the
        # PE in one (128,128) tile config -- no pipeline-restart penalty).
        qz = [[persist.tile([P, S], FP16, tag=f"qz{b}{h}", name=f"qz{b}{h}")
               for h in range(HPC)] for b in range(B)]
        kT_sb = [persist.tile([P, S], FP16, tag=f"kT{b}", name=f"kT{b}") for b in range(B)]
        ctxn = [persist.tile([P, S], FP16, tag=f"ctxn{b}", name=f"ctxn{b}") for b in range(B)]
        v_all = persist.tile([P, B * ST * VSTRIDE], FP16, tag="v_all", name="v_all")
        w_sb = persist.tile([P, 3 * DT * P], FP16, tag="w_sb", name="w_sb")
        woT_sb = persist.tile([P, D], FP16, tag="woT_sb", name="woT_sb")
        ident = persist.tile([P, P], FP16, tag="ident", name="ident")
        id8_sb = persist.tile([P, 2, P], FP8, tag="id8", name="id8")
        nc.sync.dma_start(id8_sb[:], id8[:])

        make_identity(nc, ident[:])
        nc.any.memset(v_all[:], 1.0)  # ones columns survive the v copies
        for b in range(B):
            for h in range(HPC):
                o = (1 - h) * HD  # zero the other head's half
                nc.any.memset(qz[b][h][o:o + HD, :], 0.0)
        nc.sync.dma_start(woT_sb[:], woT[:])
        nc.sync.dma_start(w_sb[:], w3[:])

        # ---- phase P: projections + v transpose ----
        with (
            tc.tile_pool(name="hp", bufs=2) as hp,
            tc.tile_pool(name="vt", bufs=8) as vtp,
            tc.tile_pool(name="psP", bufs=6, space="PSUM") as psP,
            tc.tile_pool(name="psT", bufs=2, space="PSUM") as psT,
        ):
            for b in range(B):
                h_all = hp.tile([P, DT * S], FP16, tag="h", name=f"h_{b}")
                for dt in range(DT):
                    nc.sync.dma_start(h_all[:, dt * S:(dt + 1) * S], hT[b, :, dt])
                h_sb = [h_all[:, dt * S:(dt + 1) * S] for dt in range(DT)]
                vT_tiles = []
                for p in range(3):
                    ps_qc = [
                        psP.tile([P, QCW], FP32, tag="pj", name=f"pj_{b}_{p}_{qc}")
                        for qc in range(QC)
                    ]
                    for dt in range(DT):
                        for qc in range(QC):
                            nc.tensor.matmul(
                                ps_qc[qc][:],
                                w_sb[:, (p * DT + dt) * P:(p * DT + dt + 1) * P],
                                h_sb[dt][:, qc * QCW:(qc + 1) * QCW],
                                start=(dt == 0), stop=(dt == DT - 1),
                            )
                    for qc in range(QC):
                        if p == 0:
                            for h in range(HPC):
                                hs_ = slice(h * HD, (h + 1) * HD)
                                nc.scalar.activation(
                                    qz[b][h][hs_, qc * QCW:(qc + 1) * QCW],
                                    ps_qc[qc][hs_, :],
                                    Copy, scale=1.0 / np.sqrt(HD),
                                )
                        elif p == 1:
                            nc.scalar.activation(
                                kT_sb[b][:, qc * QCW:(qc + 1) * QCW], ps_qc[qc][:], Copy)
                        else:
                            vt = vtp.tile([P, QCW], FP16, tag="v", name=f"vt_{b}_{qc}")
                            nc.vector.tensor_copy(out=vt[:], in_=ps_qc[qc][:])
                            vT_tiles.append(vt)
                # transpose vT [ch, s] -> v [s, ch] in 128x128 blocks
                for st in range(ST):
                    tp = psT.tile([P, P], FP16, tag="tr", name=f"tr_{b}_{st}")
                    src = vT_tiles[st // 4]
                    nc.tensor.transpose(tp[:], src[:, (st % 4) * P:(st % 4 + 1) * P], ident[:])
                    base = (b * ST + st) * VSTRIDE
                    nc.vector.tensor_copy(out=v_all[:, base:base + HD], in_=tp[:, 0:HD])
                    nc.vector.tensor_copy(
                        out=v_all[:, base + HD + 1:base + 2 * HD + 1], in_=tp[:, HD:2 * HD])

        # ---- phase A: attention ----
        with (
            tc.tile_pool(name="bias", bufs=ST + 8) as bp,
            tc.tile_pool(name="pr", bufs=8) as prp,
            tc.tile_pool(name="sm", bufs=4) as smp,
            tc.tile_pool(name="psS", bufs=3, space="PSUM") as psS,
            tc.tile_pool(name="psC", bufs=2, space="PSUM") as psC,
            tc.tile_pool(name="op", bufs=4) as op,
        ):
            # out-projection tiles pending emission; interleaved into the kt
            # loops so the tail phase nearly vanishes and the PE never gets a
            # multi-us idle window (HAM stays warm).
            pending_out = []

            def _emit_out():
                b, st = pending_out.pop(0)
                o_ps = psS.tile([P, 2 * QCW], FP32, tag="s", name=f"o_{b}_{st}")
                for ec in range(2):
                    nc.tensor.matmul(
                        o_ps[:, ec * QCW:(ec + 1) * QCW],
                        ctxn[b][:, st * P:(st + 1) * P],
                        woT_sb[:, ec * QCW:(ec + 1) * QCW],
                        start=True, stop=True,
                    )
                o_sb = op.tile([P, D], FP16, tag="ot", name=f"ot_{b}_{st}")
                # split the PSUM->SBUF copy across ACT and DVE
                nc.scalar.activation(o_sb[:, 0:QCW], o_ps[:, 0:QCW], Copy)
                nc.vector.tensor_copy(out=o_sb[:, QCW:D], in_=o_ps[:, QCW:D])
                nc.sync.dma_start(outp[b, st * P:(st + 1) * P, :], o_sb[:])

            for h in range(HPC):
                eb_sb = {}
                for kt in range(ST):
                    t = bp.tile([P, 2, S], FP8, tag="eb", name=f"eb_{h}_{kt}")
                    nc.sync.dma_start(t[:], eb8[h, kt])
                    eb_sb[kt] = t
                hs = slice(h * HD, (h + 1) * HD)
                for qc in range(QC):
                    ctx_ps = [
                        psC.tile([P, QCW], FP32, tag="c", name=f"ctx_{h}_{qc}_{b}")
                        for b in range(B)
                    ]
                    # software-pipelined rel inject: inject(kt) is issued one
                    # iteration ahead (between scores(kt-1) and ctx(kt-1)) so
                    # every LDWEIGHTS prefetches in the shadow of a >=216ns MM
                    # and the inject->scores PSUM drain is far separated.
                    s_tiles = {}

                    def _inject(kt):
                        s_ps = psS.tile([P, 2 * QCW], FP32, tag="s", name=f"s_{h}_{qc}_{kt}")
                        s_tiles[kt] = s_ps
                        ebs = eb_sb[kt][:, :, qc * QCW:(qc + 1) * QCW]
                        for b in range(B):
                            nc.tensor.matmul(
                                s_ps[:, b * QCW:(b + 1) * QCW],
                                id8_sb[:], ebs,
                                start=True, stop=False,
                                perf_mode=DR,
                            )

                    _inject(0)
                    for kt in range(ST):
                        s_ps = s_tiles.pop(kt)
                        for b in range(B):
                            nc.tensor.matmul(
                                s_ps[:, b * QCW:(b + 1) * QCW],
                                kT_sb[b][:, kt * P:(kt + 1) * P],
                                qz[b][h][:, qc * QCW:(qc + 1) * QCW],
                                start=False, stop=True,
                            )
                        if kt + 1 < ST:
                            _inject(kt + 1)
                        elif pending_out:
                            _emit_out()
                        pr = prp.tile([P, 2 * QCW], FP16, tag="p", name=f"p_{h}_{qc}_{kt}")
                        nc.scalar.activation(pr[:], s_ps[:], Exp)
                        for b in range(B):
                            vbase = (b * ST + kt) * VSTRIDE + h * (HD + 1)
                            nc.tensor.matmul(
                                ctx_ps[b][0:HD + 1, :],
                                v_all[:, vbase:vbase + HD + 1],
                                pr[:, b * QCW:(b + 1) * QCW],
                                start=(kt == 0), stop=(kt == ST - 1),
                            )
                        if pending_out and kt % 2 == 1:
                            _emit_out()
                    for b in range(B):
                        # evacuate PSUM first so psC recycles fast, then
                        # normalize entirely on DVE + GpSimd (ACT stays free).
                        # (recip needs its input at partition base 0.)
                        dn = smp.tile([1, QCW], FP32, tag="dn", name=f"dn_{h}_{qc}_{b}")
                        nc.vector.tensor_copy(out=dn[:], in_=ctx_ps[b][HD:HD + 1, :])
                        cs = smp.tile([HD, QCW], FP32, tag="cs", name=f"cs_{h}_{qc}_{b}")
                        nc.vector.tensor_copy(out=cs[:], in_=ctx_ps[b][0:HD, :])
                        rcp = smp.tile([1, QCW], FP32, tag="rc", name=f"rc_{h}_{qc}_{b}")
                        nc.vector.reciprocal_approx_fast(out=rcp[:], in_=dn[:])
                        bc = smp.tile([HD, QCW], FP32, tag="bc", name=f"bcs_{h}_{qc}_{b}")
                        nc.gpsimd.partition_broadcast(bc[:], rcp[:])
                        nc.vector.tensor_tensor(
                            ctxn[b][hs, qc * QCW:(qc + 1) * QCW],
                            cs[:], bc[:], MULT)
                    if h == HPC - 1:
                        # this qc's out rows are final once both heads did it
                        pending_out.extend(
                            (b, qc * (ST // QC) + i)
                            for b in range(B) for i in range(ST // QC))
            while pending_out:
                _emit_out()

        persist.release()

    nc.finalize()
    return nc


def _numpy_reference(hidden_states, attention_mask, relative_position,
                     Wq, bq, Wk, bk, Wv, bv, Wo, bo):
    Bn, Sn, Dn = hidden_states.shape
    Hn = relative_position.shape[1]
    hd = Dn // Hn
    x = hidden_states.astype(np.float64)

    def heads(t):
        return t.reshape(Bn, Sn, Hn, hd).transpose(0, 2, 1, 3)

    q = heads(x @ Wq.T.astype(np.float64) + bq)
    k = heads(x @ Wk.T.astype(np.float64) + bk)
    v = heads(x @ Wv.T.astype(np.float64) + bv)
    s = np.einsum("bhqd,bhkd->bhqk", q, k) / np.sqrt(hd)
    s = s + relative_position.astype(np.float64) + attention_mask.astype(np.float64)
    s = s - s.max(axis=-1, keepdims=True)
    p = np.exp(s)
    p /= p.sum(axis=-1, keepdims=True)
    ctx = np.einsum("bhqk,bhkd->bhqd", p, v)
    ctx = ctx.transpose(0, 2, 1, 3).reshape(Bn, Sn, Dn)
    return (ctx @ Wo.T.astype(np.float64) + bo).astype(np.float32)


def kernel(hidden_states, attention_mask, relative_position,
           Wq, bq, Wk, bk, Wv, bv, Wo, bo):
    hidden_states = np.asarray(hidden_states)
    attention_mask = np.asarray(attention_mask)
    relative_position = np.asarray(relative_position)
    Wq, bq = np.asarray(Wq), np.asarray(bq)
    Wk, bk = np.asarray(Wk), np.asarray(bk)
    Wv, bv = np.asarray(Wv), np.asarray(bv)
    Wo, bo = np.asarray(Wo), np.asarray(bo)

    # The device program folds the (always-zero) mask and qkv biases away;
    # fall back to a plain numpy path if they are ever nonzero.
    if (np.any(attention_mask) or np.any(bq) or np.any(bk) or np.any(bv)
            or hidden_states.shape != (B, S, D)):
        return _numpy_reference(hidden_states, attention_mask, relative_position,
                                Wq, bq, Wk, bk, Wv, bv, Wo, bo)

    if "nc" not in _NC_CACHE:
        _NC_CACHE["nc"] = _build_nc()
        _NC_CACHE["fp8np"] = mybir.dt.np(FP8)
    nc = _NC_CACHE["nc"]

    hT = np.ascontiguousarray(
        hidden_states.transpose(0, 2, 1).reshape(B, DT, P, S).transpose(0, 2, 1, 3)
    ).astype(np.float16)  # [B, 128, dt, S]
    rel = relative_position[0]  # [H, S, S]

    fp8 = _NC_CACHE["fp8np"]
    id8v = np.zeros((P, 2, P), fp8)
    for p in range(P):
        id8v[p, 0, p] = 1.0
        id8v[p, 1, p] = 1.0

    in_maps = []
    for c in range(NCORES):
        sl = slice(c * HPC * HD, (c + 1) * HPC * HD)
        heads = rel[c * HPC:(c + 1) * HPC]  # [HPC, S, S] (q, k)
        ebT = np.ascontiguousarray(
            heads.transpose(0, 2, 1).reshape(HPC, ST, P, S))  # [HPC, kt, p, q]
        hi = ebT.astype(fp8)
        lo = (ebT - hi.astype(np.float32)).astype(fp8)
        eb8v = np.ascontiguousarray(
            np.stack([hi, lo], axis=3))  # [HPC, ST, P, 2, S]
        w3 = np.ascontiguousarray(
            np.stack([Wq[sl].T, Wk[sl].T, Wv[sl].T])       # [3, D, 128]
            .reshape(3, DT, P, P).transpose(2, 0, 1, 3)     # [128, 3, dt, 128]
            .reshape(P, 3 * DT * P)).astype(np.float16)
        woT = np.ascontiguousarray(Wo[:, sl].T).astype(np.float16)
        in_maps.append({"hT": hT, "w3": w3, "eb8": eb8v, "id8": id8v, "woT": woT})

    res = run_bass_kernel_spmd(nc, in_maps, core_ids=list(range(NCORES)))
    _NC_CACHE["last_results"] = res

    out = np.zeros((B, S, D), np.float32)
    for c in range(NCORES):
        out += res.results[c]["outp"].astype(np.float32)
    out += bo.astype(np.float32)
    return out


# revision 39
# speedup vs baseline: 1.2860x; 1.0488x over previous
"""Trainium2 Bass kernel for nn_EnhancedAttention (16-head attention with a
full [H,S,S] additive position bias), sharded 2-heads-per-core over 8 cores.

v3 (vs v2 baseline, 333us):
  - HAM p-state fix: the attention phase starved the PE every kt-tile
    (ACT exp 1147ns > PE 852ns), keeping the PE clock-gated at 1.2GHz.
    Now half the kt-tiles "inject" the raw position bias into PSUM via an
    identity matmul (PE: +426ns/tile) so exp(s+rel) needs no DVE multiply,
    while the other half keep the exp(rel) DVE-multiply path. Average PE
    work/kt (~1170ns) slightly exceeds ACT (1147ns), keeping the PE
    saturated and warm (2.4GHz).
  - softmax denominator reciprocal moved from ACT (Ln+Exp, ~23us) to a
    single DVE custom op (reciprocal_approx_fast), broadcast in fp32.
  - PSUM: psS bufs=3 x [128,1024] (6 banks) + psC bufs=2 (2 banks) = 8.
"""

import numpy as np

import concourse.bacc as bacc
import concourse.tile as tile
import concourse.mybir as mybir
from concourse.bass_utils import run_bass_kernel_spmd
from concourse.masks import make_identity

FP16 = mybir.dt.float16
FP32 = mybir.dt.float32
FP8 = mybir.dt.float8e4
DR = mybir.MatmulPerfMode.DoubleRow
Exp = mybir.ActivationFunctionType.Exp
Ln = mybir.ActivationFunctionType.Ln
Copy = mybir.ActivationFunctionType.Copy
MULT = mybir.AluOpType.mult

P = 128
B, S, D = 2, 2048, 1024
H, HD = 16, 64
NCORES = 8
HPC = H // NCORES          # heads per core = 2
DT = D // P                # 8 d-tiles
ST = S // P                # 16 s-tiles (k tiles / out row tiles)
QC = 4                     # q chunks
QCW = S // QC              # 512
VSTRIDE = 2 * (HD + 1)     # 130: [h0 v (64) | ones | h1 v (64) | ones]


_NC_CACHE = {}


class _Bacc(bacc.Bacc):
    """Pin activations to natural_log_exp_and_others (Copy/Exp only now)."""

    def insert_act_table_loads(self):
        import bass_rust as _bass_rust
        import concourse.mybir as _mybir
        from concourse.hw_specs import get_activation_tables
        has_activation = any(
            isinstance(i, _mybir.InstActivation)
            for b in self.main_func.blocks
            for i in b.instructions
        )
        if not has_activation:
            return
        tables = []
        for name, fns in get_activation_tables(self.m.arch).items():
            tables.append((name, fns if name == "natural_log_exp_and_others" else type(fns)()))
        _bass_rust.insert_act_table_loads(self, tables)


def _build_nc():
    nc = _Bacc("TRN2", target_bir_lowering=False)

    hT = nc.dram_tensor("hT", [B, P, DT, S], FP16, kind="ExternalInput")
    w3 = nc.dram_tensor("w3", [P, 3 * DT * P], FP16, kind="ExternalInput")
    # rel bias, two forms: even k-tiles are PE-injected into PSUM before the
    # scores matmul (fp8 DoubleRow hi+lo planes summed by the identity ->
    # ~fp16 precision); odd k-tiles ship exp(rel) fp16 and multiply the
    # exp'd scores on DVE. The split balances PE vs ACT vs DVE load.
    eb8 = nc.dram_tensor("eb8", [HPC, ST // 2, P, 2, S], FP8, kind="ExternalInput")
    ebm = nc.dram_tensor("ebm", [HPC, ST // 2, P, S], FP16, kind="ExternalInput")
    id8 = nc.dram_tensor("id8", [P, 2, P], FP8, kind="ExternalInput")
    woT = nc.dram_tensor("woT", [P, D], FP16, kind="ExternalInput")
    outp = nc.dram_tensor("outp", [B, S, D], FP16, kind="ExternalOutput")

    with tile.TileContext(nc) as tc:
        # ---- persistent tiles ----
        persist = tc.alloc_tile_pool(name="persist", bufs=1)
        # per-(b, h) zero-padded q: the other head's 64 rows stay 0 so the
        # scores matmul can contract over the full 128 partitions (keeps the
        # PE in one (128,128) tile config -- no pipeline-restart penalty).
        qz = [[persist.tile([P, S], FP16, tag=f"qz{b}{h}", name=f"qz{b}{h}")
               for h in range(HPC)] for b in range(B)]
        kT_sb = [persist.tile([P, S], FP16, tag=f"kT{b}", name=f"kT{b}") for b in range(B)]
        ctxn = [persist.tile([P, S], FP16, tag=f"ctxn{b}", name=f"ctxn{b}") for b in range(B)]
        v_all = persist.tile([P, B * ST * VSTRIDE], FP16, tag="v_all", name="v_all")
        w_sb = persist.tile([P, 3 * DT * P], FP16, tag="w_sb", name="w_sb")
        woT_sb = persist.tile([P, D], FP16, tag="woT_sb", name="woT_sb")
        ident = persist.tile([P, P], FP16, tag="ident", name="ident")
        id8_sb = persist.tile([P, 2, P], FP8, tag="id8", name="id8")
        nc.sync.dma_start(id8_sb[:], id8[:])

        make_identity(nc, ident[:])
        nc.any.memset(v_all[:], 1.0)  # ones columns survive the v copies
        for b in range(B):
            for h in range(HPC):
                o = (1 - h) * HD  # zero the other head's half
                nc.any.memset(qz[b][h][o:o + HD, :], 0.0)
        nc.sync.dma_start(woT_sb[:], woT[:])
        nc.sync.dma_start(w_sb[:], w3[:])

        # bias pool outlives phase P so the h0 rel tiles stream in during the
        # projections (they must be resident the moment attention starts).
        bp = tc.alloc_tile_pool(name="bias", bufs=10)
        eb_sb = {h: {} for h in range(HPC)}

        def _load_eb(h):
            for kt in range(ST):
                if kt % 2 == 0:
                    t = bp.tile([P, 2, S], FP8, tag="eb8", name=f"eb_{h}_{kt}")
                    nc.sync.dma_start(t[:], eb8[h, kt // 2])
                else:
                    t = bp.tile([P, S], FP16, tag="ebm", name=f"eb_{h}_{kt}")
                    nc.sync.dma_start(t[:], ebm[h, kt // 2])
                eb_sb[h][kt] = t

        # ---- phase P: projections + v transpose ----
        with (
            tc.tile_pool(name="hp", bufs=2) as hp,
            tc.tile_pool(name="vt", bufs=8) as vtp,
            tc.tile_pool(name="psP", bufs=6, space="PSUM") as psP,
            tc.tile_pool(name="psT", bufs=2, space="PSUM") as psT,
        ):
            h_alls = []
            for b in range(B):
                h_all = hp.tile([P, DT * S], FP16, tag="h", name=f"h_{b}")
                for dt in range(DT):
                    nc.sync.dma_start(h_all[:, dt * S:(dt + 1) * S], hT[b, :, dt])
                h_alls.append(h_all)
            _load_eb(0)
            for b in range(B):
                h_all = h_alls[b]
                h_sb = [h_all[:, dt * S:(dt + 1) * S] for dt in range(DT)]
                vT_tiles = []
                for p in range(3):
                    ps_qc = [
                        psP.tile([P, QCW], FP32, tag="pj", name=f"pj_{b}_{p}_{qc}")
                        for qc in range(QC)
                    ]
                    for dt in range(DT):
                        for qc in range(QC):
                            nc.tensor.matmul(
                                ps_qc[qc][:],
                                w_sb[:, (p * DT + dt) * P:(p * DT + dt + 1) * P],
                                h_sb[dt][:, qc * QCW:(qc + 1) * QCW],
                                start=(dt == 0), stop=(dt == DT - 1),
                            )
                    for qc in range(QC):
                        if p == 0:
                            for h in range(HPC):
                                hs_ = slice(h * HD, (h + 1) * HD)
                                nc.scalar.activation(
                                    qz[b][h][hs_, qc * QCW:(qc + 1) * QCW],
                                    ps_qc[qc][hs_, :],
                                    Copy, scale=1.0 / np.sqrt(HD),
                                )
                        elif p == 1:
                            nc.scalar.activation(
                                kT_sb[b][:, qc * QCW:(qc + 1) * QCW], ps_qc[qc][:], Copy)
                        else:
                            vt = vtp.tile([P, QCW], FP16, tag="v", name=f"vt_{b}_{qc}")
                            nc.vector.tensor_copy(out=vt[:], in_=ps_qc[qc][:])
                            vT_tiles.append(vt)
                # transpose vT [ch, s] -> v [s, ch] in 128x128 blocks
                for st in range(ST):
                    tp = psT.tile([P, P], FP16, tag="tr", name=f"tr_{b}_{st}")
                    src = vT_tiles[st // 4]
                    nc.tensor.transpose(tp[:], src[:, (st % 4) * P:(st % 4 + 1) * P], ident[:])
                    base = (b * ST + st) * VSTRIDE
                    nc.vector.tensor_copy(out=v_all[:, base:base + HD], in_=tp[:, 0:HD])
                    nc.vector.tensor_copy(
                        out=v_all[:, base + HD + 1:base + 2 * HD + 1], in_=tp[:, HD:2 * HD])

        # ---- phase A: attention ----
        with (
            tc.tile_pool(name="pr", bufs=6) as prp,
            tc.tile_pool(name="sm", bufs=3) as smp,
            tc.tile_pool(name="psS", bufs=3, space="PSUM") as psS,
            tc.tile_pool(name="psC", bufs=2, space="PSUM") as psC,
            tc.tile_pool(name="op", bufs=4) as op,
        ):
            # out-projection tiles pending emission; interleaved into the kt
            # loops so the tail phase nearly vanishes and the PE never gets a
            # multi-us idle window (HAM stays warm).
            pending_out = []

            def _emit_out():
                b, st = pending_out.pop(0)
                o_ps = psS.tile([P, 2 * QCW], FP32, tag="s", name=f"o_{b}_{st}")
                for ec in range(2):
                    nc.tensor.matmul(
                        o_ps[:, ec * QCW:(ec + 1) * QCW],
                        ctxn[b][:, st * P:(st + 1) * P],
                        woT_sb[:, ec * QCW:(ec + 1) * QCW],
                        start=True, stop=True,
                    )
                o_sb = op.tile([P, D], FP16, tag="ot", name=f"ot_{b}_{st}")
                # split the PSUM->SBUF copy across ACT and DVE
                nc.scalar.activation(o_sb[:, 0:QCW], o_ps[:, 0:QCW], Copy)
                nc.vector.tensor_copy(out=o_sb[:, QCW:D], in_=o_ps[:, QCW:D])
                nc.sync.dma_start(outp[b, st * P:(st + 1) * P, :], o_sb[:])

            for h in range(HPC):
                if h > 0:
                    _load_eb(h)
                hs = slice(h * HD, (h + 1) * HD)
                for qc in range(QC):
                    ctx_ps = [
                        psC.tile([P, QCW], FP32, tag="c", name=f"ctx_{h}_{qc}_{b}")
                        for b in range(B)
                    ]
                    # software-pipelined rel inject: inject(kt) is issued one
                    # iteration ahead (between scores(kt-1) and ctx(kt-1)) so
                    # every LDWEIGHTS prefetches in the shadow of a >=216ns MM
                    # and the inject->scores PSUM drain is far separated.
                    s_tiles = {}

                    def _inject(kt):
                        s_ps = psS.tile([P, 2 * QCW], FP32, tag="s", name=f"s_{h}_{qc}_{kt}")
                        s_tiles[kt] = s_ps
                        if kt % 2 == 1:
                            return  # odd kt: DVE-multiply path, no inject
                        ebs = eb_sb[h][kt][:, :, qc * QCW:(qc + 1) * QCW]
                        for b in range(B):
                            nc.tensor.matmul(
                                s_ps[:, b * QCW:(b + 1) * QCW],
                                id8_sb[:], ebs,
                                start=True, stop=False,
                                perf_mode=DR,
                            )

                    _inject(0)
                    for kt in range(ST):
                        inj = kt % 2 == 0
                        s_ps = s_tiles.pop(kt)
                        for b in range(B):
                            nc.tensor.matmul(
                                s_ps[:, b * QCW:(b + 1) * QCW],
                                kT_sb[b][:, kt * P:(kt + 1) * P],
                                qz[b][h][:, qc * QCW:(qc + 1) * QCW],
                                start=not inj, stop=True,
                            )
                        if kt + 1 < ST:
                            _inject(kt + 1)
                        elif pending_out:
                            _emit_out()
                        pr = prp.tile([P, 2 * QCW], FP16, tag="p", name=f"p_{h}_{qc}_{kt}")
                        nc.scalar.activation(pr[:], s_ps[:], Exp)
                        if not inj:
                            ebs = eb_sb[h][kt][:, qc * QCW:(qc + 1) * QCW]
                            for b in range(B):
                                nc.vector.tensor_tensor(
                                    pr[:, b * QCW:(b + 1) * QCW],
                                    pr[:, b * QCW:(b + 1) * QCW],
                                    ebs, MULT)
                        for b in range(B):
                            vbase = (b * ST + kt) * VSTRIDE + h * (HD + 1)
                            nc.tensor.matmul(
                                ctx_ps[b][0:HD + 1, :],
                                v_all[:, vbase:vbase + HD + 1],
                                pr[:, b * QCW:(b + 1) * QCW],
                                start=(kt == 0), stop=(kt == ST - 1),
                            )
                        if pending_out and kt % 2 == 1:
                            _emit_out()
                    for b in range(B):
                        # evacuate PSUM first so psC recycles fast, then
                        # normalize entirely on DVE + GpSimd (ACT stays free).
                        # (recip needs its input at partition base 0.)
                        dn = smp.tile([1, QCW], FP32, tag="dn", name=f"dn_{h}_{qc}_{b}")
                        nc.vector.tensor_copy(out=dn[:], in_=ctx_ps[b][HD:HD + 1, :])
                        cs = smp.tile([HD, QCW], FP32, tag="cs", name=f"cs_{h}_{qc}_{b}")
                        nc.vector.tensor_copy(out=cs[:], in_=ctx_ps[b][0:HD, :])
                        rcp = smp.tile([1, QCW], FP32, tag="rc", name=f"rc_{h}_{qc}_{b}")
                        nc.vector.reciprocal_approx_fast(out=rcp[:], in_=dn[:])
                        bc = smp.tile([HD, QCW], FP32, tag="bc", name=f"bcs_{h}_{qc}_{b}")
                        nc.gpsimd.partition_broadcast(bc[:], rcp[:])
                        nc.vector.tensor_tensor(
                            ctxn[b][hs, qc * QCW:(qc + 1) * QCW],
                            cs[:], bc[:], MULT)
                    if h == HPC - 1:
                        # this qc's out rows are final once both heads did it
                        pending_out.extend(
                            (b, qc * (ST // QC) + i)
                            for b in range(B) for i in range(ST // QC))
            while pending_out:
                _emit_out()

        bp.release()
        persist.release()

    nc.finalize()
    return nc


def _numpy_reference(hidden_states, attention_mask, relative_position,
                     Wq, bq, Wk, bk, Wv, bv, Wo, bo):
    Bn, Sn, Dn = hidden_states.shape
    Hn = relative_position.shape[1]
    hd = Dn // Hn
    x = hidden_states.astype(np.float64)

    def heads(t):
        return t.reshape(Bn, Sn, Hn, hd).transpose(0, 2, 1, 3)

    q = heads(x @ Wq.T.astype(np.float64) + bq)
    k = heads(x @ Wk.T.astype(np.float64) + bk)
    v = heads(x @ Wv.T.astype(np.float64) + bv)
    s = np.einsum("bhqd,bhkd->bhqk", q, k) / np.sqrt(hd)
    s = s + relative_position.astype(np.float64) + attention_mask.astype(np.float64)
    s = s - s.max(axis=-1, keepdims=True)
    p = np.exp(s)
    p /= p.sum(axis=-1, keepdims=True)
    ctx = np.einsum("bhqk,bhkd->bhqd", p, v)
    ctx = ctx.transpose(0, 2, 1, 3).reshape(Bn, Sn, Dn)
    return (ctx @ Wo.T.astype(np.float64) + bo).astype(np.float32)


def kernel(hidden_states, attention_mask, relative_position,
           Wq, bq, Wk, bk, Wv, bv, Wo, bo):
    hidden_states = np.asarray(hidden_states)
    attention_mask = np.asarray(attention_mask)
    relative_position = np.asarray(relative_position)
    Wq, bq = np.asarray(Wq), np.asarray(bq)
    Wk, bk = np.asarray(Wk), np.asarray(bk)
    Wv, bv = np.asarray(Wv), np.asarray(bv)
    Wo, bo = np.asarray(Wo), np.asarray(bo)

    # The device program folds the (always-zero) mask and qkv biases away;
    # fall back to a plain numpy path if they are ever nonzero.
    if (np.any(attention_mask) or np.any(bq) or np.any(bk) or np.any(bv)
            or hidden_states.shape != (B, S, D)):
        return _numpy_reference(hidden_states, attention_mask, relative_position,
                                Wq, bq, Wk, bk, Wv, bv, Wo, bo)

    if "nc" not in _NC_CACHE:
        _NC_CACHE["nc"] = _build_nc()
        _NC_CACHE["fp8np"] = mybir.dt.np(FP8)
    nc = _NC_CACHE["nc"]

    hT = np.ascontiguousarray(
        hidden_states.transpose(0, 2, 1).reshape(B, DT, P, S).transpose(0, 2, 1, 3)
    ).astype(np.float16)  # [B, 128, dt, S]
    rel = relative_position[0]  # [H, S, S]

    fp8 = _NC_CACHE["fp8np"]
    id8v = np.zeros((P, 2, P), fp8)
    for p in range(P):
        id8v[p, 0, p] = 1.0
        id8v[p, 1, p] = 1.0

    in_maps = []
    for c in range(NCORES):
        sl = slice(c * HPC * HD, (c + 1) * HPC * HD)
        heads = rel[c * HPC:(c + 1) * HPC]  # [HPC, S, S] (q, k)
        ebT = np.ascontiguousarray(
            heads.transpose(0, 2, 1).reshape(HPC, ST, P, S))  # [HPC, kt, p, q]
        ebe = ebT[:, 0::2]  # even k-tiles: raw rel, fp8 hi+lo inject
        hi = ebe.astype(fp8)
        lo = (ebe - hi.astype(np.float32)).astype(fp8)
        eb8v = np.ascontiguousarray(np.stack([hi, lo], axis=3))
        ebmv = np.ascontiguousarray(np.exp(ebT[:, 1::2]).astype(np.float16))
        w3 = np.ascontiguousarray(
            np.stack([Wq[sl].T, Wk[sl].T, Wv[sl].T])       # [3, D, 128]
            .reshape(3, DT, P, P).transpose(2, 0, 1, 3)     # [128, 3, dt, 128]
            .reshape(P, 3 * DT * P)).astype(np.float16)
        woT = np.ascontiguousarray(Wo[:, sl].T).astype(np.float16)
        in_maps.append({"hT": hT, "w3": w3, "eb8": eb8v, "ebm": ebmv,
                        "id8": id8v, "woT": woT})

    res = run_bass_kernel_spmd(nc, in_maps, core_ids=list(range(NCORES)))
    _NC_CACHE["last_results"] = res

    out = np.zeros((B, S, D), np.float32)
    for c in range(NCORES):
        out += res.results[c]["outp"].astype(np.float32)
    out += bo.astype(np.float32)
    return out
